# revision 1
# baseline (speedup 1.0000x reference)
"""MultiHeadAttention kernel for 8 TRN2 NeuronCores (Bass/Tile, SPMD).

Sharding: core c = (batch b, head-group g) with b = c//4, g = c%4.
Each core computes heads [4g, 4g+4) of batch b:
  Q/K/V column-parallel projections (256 hidden cols per core),
  full attention over its 4 heads (S=2048, head_dim=64),
  row-parallel output projection -> partial [S, EMB] output.
Host gathers: out[b] = sum_g partial[b,g] + bo  (the TP all-reduce).

Storage dtype is bf16 (inputs pre-cast on host, halves HBM traffic);
every accumulation happens in fp32 PSUM. Softmax skips the
max-subtraction (scores are O(1) here), and the denominator comes free
from a ones-column appended to V in the PV matmul. Normalization uses
reciprocal + gpsimd partition_broadcast. DMAs are batched into few
large strided transfers (HWDGE setup is ~625ns of serialized time per
dma_start), ordered so attention can start as soon as the first
q/k/v column-blocks land.
"""

from contextlib import ExitStack

import ml_dtypes
import numpy as np

import concourse.bass as bass
import concourse.mybir as mybir
import concourse.tile as tile
from concourse.masks import make_identity
from concourse import bacc
from concourse.bass_utils import run_bass_kernel_spmd

F32 = mybir.dt.float32
BF16 = mybir.dt.bfloat16
AF = mybir.ActivationFunctionType
NPBF16 = np.dtype(ml_dtypes.bfloat16)

S = 2048          # sequence length
E = 1024          # embedding dim
HC = 256          # hidden cols per core (= 4 heads * 64)
NH = 4            # heads per core
HD = 64           # head dim
ET = E // 128     # e-tiles (8)
QC = S // 512     # 512-wide q chunks (4)
KT = S // 128     # 128-wide k tiles (16)
SCALE = 1.0 / 8.0  # 1/sqrt(HD)

_CACHE: dict = {}


def _emit(nc, tc, io, stop_after=None):
    xqT, xkT, xvT = io["xqT"], io["xkT"], io["xvT"]
    wq, wk, wv, wo = io["wq"], io["wk"], io["wv"], io["wo"]
    b3 = io["b3"]
    outT = io["outT"]

    with ExitStack() as ctx:
        consts = ctx.enter_context(tc.tile_pool(name="consts", bufs=1))
        big = ctx.enter_context(tc.tile_pool(name="big", bufs=1))
        xp = ctx.enter_context(tc.tile_pool(name="xp", bufs=8))
        ptp = ctx.enter_context(tc.tile_pool(name="pt", bufs=12))
        smal = ctx.enter_context(tc.tile_pool(name="small", bufs=3))
        outp = ctx.enter_context(tc.tile_pool(name="outstage", bufs=3))
        psA = ctx.enter_context(tc.tile_pool(name="psA", bufs=2, space="PSUM"))
        psO = ctx.enter_context(tc.tile_pool(name="psO", bufs=2, space="PSUM"))

        # ---- weights / biases to SBUF (one DMA each) ----
        wq_sb = consts.tile([128, ET, HC], BF16, tag="wq")   # [p, t, d]
        wk_sb = consts.tile([128, ET, HC], BF16, tag="wk")
        wv_sb = consts.tile([128, ET, HC], BF16, tag="wv")
        wo_sb = consts.tile([128, 2, E], BF16, tag="wo")     # [p, t, n]
        b3_sb = consts.tile([128, 4], F32, tag="b3")  # bq|bk, 2 cols each

        # ---- persistent activations ----
        QT_sb = big.tile([128, 2 * S], BF16, tag="QT")   # [p, dc*2048+s] : Q^T
        KT_sb = big.tile([128, 2 * S], BF16, tag="KT")
        V_sb = big.tile([128, KT * NH * (HD + 1)], BF16, tag="V")
        Vv = V_sb[:].rearrange("p (k h j) -> p k h j", k=KT, h=NH)
        OcatT = big.tile([128, 2 * S], BF16, tag="OT")   # [p, t*2048+s]

        # cols 0:64 = V, col 64 = ones (softmax denominator), rest unused
        nc.vector.memset(Vv[:, :, :, HD:HD + 1], 1.0)
        ident = consts.tile([128, 128], BF16, tag="ident")
        make_identity(nc, ident[:])

        # warm the ACT exp table at t~0 (the lazy ACT_TABLE_LOAD is ~1.3us
        # and otherwise lands in the first-exp critical path)
        warm = smal.tile([1, 2], F32, tag="warm")
        nc.vector.memset(warm[:], 0.0)
        nc.scalar.activation(warm[:], warm[:], AF.Exp)

        def qk_dma(xT, qc):
            xb = xp.tile([128, ET, 512], BF16, tag="xblk", name="xblk")
            nc.sync.dma_start(
                xb[:],
                xT[:, qc * 512:(qc + 1) * 512].rearrange("(t p) s -> p t s", p=128))
            return xb

        def proj_qk(xT, w_sb, boff, dst, qc, xb=None):
            if xb is None:
                xb = qk_dma(xT, qc)
            ps = [psA.tile([128, 512], F32, tag="A", name=f"ps{qc}_{dc}")
                  for dc in range(2)]
            for t in range(ET):
                for dc in range(2):
                    nc.tensor.matmul(
                        ps[dc][:],
                        w_sb[:, t, dc * 128:(dc + 1) * 128],
                        xb[:, t, :],
                        start=(t == 0), stop=(t == ET - 1))
            for dc in range(2):
                nc.vector.tensor_scalar_add(
                    dst[:, dc * S + qc * 512: dc * S + (qc + 1) * 512],
                    ps[dc][:], b3_sb[:, boff + dc: boff + dc + 1])

        def v_dma(sc):
            xb = xp.tile([128, ET, 512], BF16, tag="xblk", name="xblk")
            nc.sync.dma_start(
                xb[:],
                xvT[:, sc * 512:(sc + 1) * 512].rearrange("(t p) s -> p t s", p=128))
            return xb

        def proj_v(sc, xb=None):
            if xb is None:
                xb = v_dma(sc)
            for half in range(2):
                psv = psA.tile([128, 2, HC], F32, tag="A", name=f"psv{sc}_{half}")
                for stl in range(2):
                    # 2 s-subtiles share this PSUM bank: first starts the
                    # zero region, second stops it
                    st = 2 * half + stl
                    for t in range(ET):
                        nc.tensor.matmul(
                            psv[:, stl],
                            xb[:, t, st * 128:(st + 1) * 128],
                            wv_sb[:, t, :],
                            start=(t == 0 and stl == 0),
                            stop=(t == ET - 1 and stl == 1))
                # bv is NOT added here: it is applied after softmax
                # normalization (O = P@V0/denom + bv), see the host gather.
                nc.vector.tensor_copy(
                    Vv[:, sc * 4 + 2 * half: sc * 4 + 2 * half + 2, :, 0:HD],
                    psv[:].rearrange("p s (h j) -> p s h j", h=NH))

        def attn_s(h, qh, kts, pts):
            # scores + exp only (no V dependency -- emitted before V-proj
            # during the ramp so the in-order PE/ACT streams don't stall
            # behind the xv DMA)
            dch, po = h // 2, 64 * (h % 2)
            for kt in kts:
                pss = psA.tile([128, 1024], F32, tag="A", name=f"pss{h}_{qh}_{kt}")
                for j in range(2):
                    qoff = dch * S + (qh * 2 + j) * 512
                    nc.tensor.matmul(
                        pss[:, j * 512:(j + 1) * 512],
                        KT_sb[po:po + 64, dch * S + kt * 128: dch * S + (kt + 1) * 128],
                        QT_sb[po:po + 64, qoff:qoff + 512],
                        start=True, stop=True)
                pt = ptp.tile([128, 1024], BF16, tag="pt", name=f"pt{h}_{qh}_{kt}")
                nc.scalar.activation(pt[:], pss[:], AF.Exp, scale=SCALE)
                pts[(h, qh, kt)] = pt

        def attn_pv(h, qh, pso, kts, pts):
            # O in natural [q, d] layout: 8 q-subtiles of [128, 64+denom]
            # at 128-col strides inside pso (each fits one PSUM bank).
            # start zeroes a whole 2KB PSUM bank ("zero region"), so only
            # the first q-subtile per bank starts the group and only the
            # last one stops it (4 subtiles share each bank).
            for kt in kts:
                pt = pts.pop((h, qh, kt))
                for qt in range(8):
                    nc.tensor.matmul(
                        pso[:, qt * 128: qt * 128 + HD + 1],
                        pt[:, qt * 128:(qt + 1) * 128],
                        Vv[:, kt, h, 0:HD + 1],
                        start=(kt == 0 and qt % 4 == 0),
                        stop=(kt == KT - 1 and qt % 4 == 3))

        def attn_kts(h, qh, pso, kts):
            pts = {}
            for kt in kts:
                attn_s(h, qh, [kt], pts)
                attn_pv(h, qh, pso, [kt], pts)

        def attn_norm(h, qh, pso):
            # O/denom per q (partition) -> transpose [128q,64d] -> OcatT
            dch, po = h // 2, 64 * (h % 2)
            psv8 = pso[:].rearrange("p (t c) -> p t c", t=8)
            recip = smal.tile([128, 8, 1], F32, tag="recip", name=f"rc{h}{qh}")
            nc.vector.reciprocal(recip[:], psv8[:, :, HD:HD + 1])
            tmpb = smal.tile([128, 8, HD], BF16, tag="tmp", name=f"tm{h}{qh}")
            for qt in range(8):
                nc.vector.tensor_scalar_mul(
                    tmpb[:, qt], psv8[:, qt, 0:HD], recip[:, qt, 0:1])
            tp = psO.tile([HD, 1024], BF16, tag="O", name=f"tp{h}{qh}")
            for qt in range(8):
                nc.tensor.transpose(
                    tp[:, qt * 128:(qt + 1) * 128], tmpb[:, qt], ident[:])
            nc.vector.tensor_copy(
                OcatT[po:po + 64, dch * S + qh * 1024: dch * S + (qh + 1) * 1024],
                tp[:])

        def attn_phase(h, qh, mid=None):
            pso = psO.tile([128, 1024], F32, tag="O", name=f"pso{h}_{qh}")
            attn_kts(h, qh, pso, range(KT))
            if mid is not None:
                mid()
            attn_norm(h, qh, pso)

        def outproj(qh, ecs=None):
            # outT cols [qh*1024, qh*1024+1024) from OcatT (needs all heads' qh)
            for ec in (range(ET) if ecs is None else ecs):
                stg = outp.tile([128, 1024], BF16, tag="stg", name=f"stg{ec}_{qh}")
                po2 = psO.tile([128, 1024], F32, tag="O", name=f"po{ec}_{qh}")
                for jq, qc in enumerate((2 * qh, 2 * qh + 1)):
                    for t in range(2):
                        nc.tensor.matmul(
                            po2[:, jq * 512:(jq + 1) * 512],
                            wo_sb[:, t, ec * 128:(ec + 1) * 128],
                            OcatT[:, t * S + qc * 512: t * S + (qc + 1) * 512],
                            start=(t == 0), stop=(t == 1))
                if qh == 1 and ec % 2 == 1:
                    # tail only: ACT is idle there; alternating engines
                    # drains the tail 2x faster. (Never mid-attention,
                    # where ACT is the bottleneck.)
                    nc.scalar.activation(stg[:], po2[:], AF.Identity)
                else:
                    nc.vector.tensor_copy(stg[:], po2[:])
                nc.sync.dma_start(
                    outT[ec * 128:(ec + 1) * 128, qh * 1024:(qh + 1) * 1024], stg[:])

        # ---- skewed-pipeline schedule ----
        # One flat stream of (head, q-half, kt) tasks. S+exp emission leads
        # the PV emission by LAG tasks so phase-boundary chains (PV kt15 ->
        # recip -> mul -> transpose -> next S) never stall the ACT engine.
        # The ramp interleaves h0/h1 qh0 per k/v DMA group (two heads of
        # exp work per group matches the DMA feed rate), with the k/v
        # projections hooked into the stream at the positions where the
        # in-order PE/ACT/DVE streams want them.
        nc.sync.dma_start(wq_sb[:], wq.rearrange("(t p) d -> p t d", p=128))
        nc.sync.dma_start(b3_sb[:], b3[:])
        proj_qk(xqT, wq_sb, 0, QT_sb, 0)
        nc.sync.dma_start(wk_sb[:], wk.rearrange("(t p) d -> p t d", p=128))
        proj_qk(xqT, wq_sb, 0, QT_sb, 1)

        tasks = []
        before_s = {}
        before_pv = {}
        # ramp: h0/h1 alternate per kt-group
        for grp in range(4):
            for h in (0, 1):
                for kt in range(4 * grp, 4 * grp + 4):
                    tasks.append((h, 0, kt))
            if grp == 0:
                before_s[(0, 0, 0)] = [
                    lambda: proj_qk(xkT, wk_sb, 2, KT_sb, 0)]
                before_pv[(0, 0, 0)] = [
                    lambda: nc.sync.dma_start(
                        wv_sb[:], wv.rearrange("(t p) d -> p t d", p=128)),
                    lambda: proj_v(0)]
            else:
                before_s.setdefault((1, 0, 4 * (grp - 1) + 2), []).append(
                    lambda g=grp: proj_qk(xkT, wk_sb, 2, KT_sb, g))
                before_pv[(0, 0, 4 * grp)] = [lambda g=grp: proj_v(g)]
        # steady qh0 phases
        for h in (2, 3):
            tasks += [(h, 0, kt) for kt in range(KT)]
        # qh1 phases
        for h in (0, 1, 2, 3):
            tasks += [(h, 1, kt) for kt in range(KT)]

        def late_dmas():
            xb23["xb2"] = qk_dma(xqT, 2)
            xb23["xb3"] = qk_dma(xqT, 3)
            nc.sync.dma_start(wo_sb[:], wo.rearrange("(t p) n -> p t n", p=128))

        xb23 = {}
        before_s[(2, 0, 0)] = [late_dmas]
        before_s[(3, 0, 8)] = [
            lambda: proj_qk(xqT, wq_sb, 0, QT_sb, 2, xb=xb23["xb2"])]
        before_s[(0, 1, 0)] = [
            lambda: proj_qk(xqT, wq_sb, 0, QT_sb, 3, xb=xb23["xb3"])]
        # qh=0 output projection interleaved under the qh=1 phases
        for h in (0, 1, 2, 3):
            before_pv.setdefault((h, 1, KT - 1), []).append(
                lambda h=h: outproj(0, ecs=range(2 * h, 2 * h + 2)))

        LAG = 4
        pts = {}
        psos = {}

        def run_pv(task):
            h, qh, kt = task
            if kt == 0:
                psos[(h, qh)] = psO.tile(
                    [128, 1024], F32, tag="O", name=f"pso{h}_{qh}")
            for fn in before_pv.get(task, []):
                fn()
            attn_pv(h, qh, psos[(h, qh)], [kt], pts)
            if kt == KT - 1:
                attn_norm(h, qh, psos.pop((h, qh)))

        for i, task in enumerate(tasks):
            for fn in before_s.get(task, []):
                fn()
            attn_s(*task[:2], [task[2]], pts)
            if i >= LAG:
                run_pv(tasks[i - LAG])
        for task in tasks[-LAG:]:
            run_pv(task)
        if stop_after in ("qk", "attn"):
            return
        outproj(1)


def build_program(stop_after=None, reps=1):
    nc = bacc.Bacc("TRN2", target_bir_lowering=False, debug=False, num_devices=8)
    io = {
        "xqT": nc.dram_tensor("xqT", [E, S], BF16, kind="ExternalInput").ap(),
        "xkT": nc.dram_tensor("xkT", [E, S], BF16, kind="ExternalInput").ap(),
        "xvT": nc.dram_tensor("xvT", [E, S], BF16, kind="ExternalInput").ap(),
        "wq": nc.dram_tensor("wq", [E, HC], BF16, kind="ExternalInput").ap(),
        "wk": nc.dram_tensor("wk", [E, HC], BF16, kind="ExternalInput").ap(),
        "wv": nc.dram_tensor("wv", [E, HC], BF16, kind="ExternalInput").ap(),
        "wo": nc.dram_tensor("wo", [HC, E], BF16, kind="ExternalInput").ap(),
        "b3": nc.dram_tensor("b3", [128, 4], F32, kind="ExternalInput").ap(),
        "outT": nc.dram_tensor("outT", [E, S], BF16, kind="ExternalOutput").ap(),
    }
    with tile.TileContext(nc) as tc:
        for _ in range(reps):
            _emit(nc, tc, io, stop_after=stop_after)
    nc.compile()
    return nc


def make_in_maps(q, k, v, Wq, bq, Wk, bk, Wv, bv, Wo, bo):
    q, k, v = (np.asarray(x, np.float32) for x in (q, k, v))
    Wq, Wk, Wv, Wo = (np.asarray(x, np.float32) for x in (Wq, Wk, Wv, Wo))
    bq, bk, bv = (np.asarray(x, np.float32) for x in (bq, bk, bv))
    xT = {b: {n: np.ascontiguousarray(a[b].T).astype(NPBF16)
              for n, a in (("xqT", q), ("xkT", k), ("xvT", v))}
          for b in range(2)}
    in_maps = []
    for c in range(8):
        b, g = divmod(c, 4)
        cs = slice(g * HC, (g + 1) * HC)
        b3 = np.stack([bq[cs].reshape(2, 128)[0], bq[cs].reshape(2, 128)[1],
                       bk[cs].reshape(2, 128)[0], bk[cs].reshape(2, 128)[1]],
                      axis=1)
        in_maps.append({
            "xqT": xT[b]["xqT"], "xkT": xT[b]["xkT"], "xvT": xT[b]["xvT"],
            "wq": np.ascontiguousarray(Wq[:, cs]).astype(NPBF16),
            "wk": np.ascontiguousarray(Wk[:, cs]).astype(NPBF16),
            "wv": np.ascontiguousarray(Wv[:, cs]).astype(NPBF16),
            "wo": np.ascontiguousarray(Wo[cs, :]).astype(NPBF16),
            "b3": np.ascontiguousarray(b3, np.float32),
        })
    return in_maps


def kernel(q, k, v, Wq, bq, Wk, bk, Wv, bv, Wo, bo):
    if "nc" not in _CACHE:
        _CACHE["nc"] = build_program()
    nc = _CACHE["nc"]
    in_maps = make_in_maps(q, k, v, Wq, bq, Wk, bk, Wv, bv, Wo, bo)
    res = run_bass_kernel_spmd(nc, in_maps, list(range(8))).results
    bo = np.asarray(bo, np.float32)
    bv = np.asarray(bv, np.float32)
    Wo = np.asarray(Wo, np.float32)
    extra = bv @ Wo + bo  # bv folds through the output projection
    out = np.empty((2, S, E), np.float32)
    for b in range(2):
        acc = res[4 * b]["outT"].astype(np.float32)
        for g in range(1, 4):
            acc += res[4 * b + g]["outT"].astype(np.float32)
        out[b] = acc.T + extra
    return out



# revision 30
# speedup vs baseline: 1.1843x; 1.1843x over previous
"""MultiHeadAttention kernel for 8 TRN2 NeuronCores (Bass/Tile, SPMD).

Sharding: core c = (batch b, head-group g) with b = c//4, g = c%4.
Each core computes heads [4g, 4g+4) of batch b:
  Q/K/V column-parallel projections (256 hidden cols per core),
  full attention over its 4 heads (S=2048, head_dim=64),
  row-parallel output projection -> partial [S, EMB] output.
Host gathers: out[b] = sum_g partial[b,g] + bo  (the TP all-reduce).

Storage dtype is bf16; accumulation in fp32 PSUM. Softmax skips the
max-subtraction (scores are O(1)); the denominator comes free from a
ones-column appended to V in the PV matmul.

Schedule (v2): the ACT engine (exp over all S^2 scores) is the binding
resource (~133us busy) with PE nearly equal (~140us). All input DMAs
are issued up front in one priority-ordered stream on the serial
~360GB/s HBM pipe. Projections and the output projection are chopped
into ~1024-cycle micro-chunks interleaved between (scores, exp, PV)
tasks so the in-order PE stream never starves ACT for more than the
one-tile PSUM lookahead. PSUM: scores double-buffer 4 banks, PV
accumulator 2, transpose 1, proj/outproj scratch 1.
"""

from contextlib import ExitStack

import ml_dtypes
import numpy as np

import concourse.bass as bass
import concourse.mybir as mybir
import concourse.tile as tile
from concourse.masks import make_identity
from concourse import bacc
from concourse.bass_utils import run_bass_kernel_spmd

F32 = mybir.dt.float32
BF16 = mybir.dt.bfloat16
F8 = mybir.dt.float8e4
AF = mybir.ActivationFunctionType
NPBF16 = np.dtype(ml_dtypes.bfloat16)
NPF8 = np.dtype(ml_dtypes.float8_e4m3)

S = 2048          # sequence length
E = 1024          # embedding dim
HC = 256          # hidden cols per core (= 4 heads * 64)
NH = 4            # heads per core
HD = 64           # head dim
ET = E // 128     # e-tiles (8)
KT = S // 128     # 128-wide k tiles (16)
SCALE = 1.0 / 8.0  # 1/sqrt(HD)

LAG = 4           # PV trails the scores/exp stream by this many tasks
PTS_CAP = 14      # max outstanding exp tiles awaiting PV

_CACHE: dict = {}


def _emit(nc, tc, io):
    xqT, xkT, xvT = io["xqT"], io["xkT"], io["xvT"]
    wq, wk, wv, wo = io["wq"], io["wk"], io["wv"], io["wo"]
    b3 = io["b3"]
    outT = io["outT"]

    with ExitStack() as ctx:
        consts = ctx.enter_context(tc.tile_pool(name="consts", bufs=1))
        big = ctx.enter_context(tc.tile_pool(name="big", bufs=1))
        xp = ctx.enter_context(tc.tile_pool(name="xp", bufs=10))
        ptp = ctx.enter_context(tc.tile_pool(name="pt", bufs=PTS_CAP + 3))
        smal = ctx.enter_context(tc.tile_pool(name="small", bufs=3))
        tmb = ctx.enter_context(tc.tile_pool(name="tmb", bufs=2))
        outp = ctx.enter_context(tc.tile_pool(name="outstage", bufs=4))
        psS = ctx.enter_context(tc.tile_pool(name="psS", bufs=2, space="PSUM"))
        psV = ctx.enter_context(tc.tile_pool(name="psV", bufs=1, space="PSUM"))
        psT = ctx.enter_context(tc.tile_pool(name="psT", bufs=1, space="PSUM"))
        psM = ctx.enter_context(tc.tile_pool(name="psM", bufs=1, space="PSUM"))

        # ---- SBUF destinations ----
        wq_sb = consts.tile([128, ET, HC], BF16, tag="wq")   # [p, t, d]
        wk_sb = consts.tile([128, ET, HC], BF16, tag="wk")
        wv_sb = consts.tile([128, ET, HC], BF16, tag="wv")
        wo_sb = consts.tile([128, 2, E], BF16, tag="wo")     # [p, t, n]
        b3_sb = consts.tile([128, 4], F32, tag="b3")  # bq|bk, 2 cols each

        QT_sb = big.tile([128, 2 * S], BF16, tag="QT")   # [p, dc*2048+s]
        KT_sb = big.tile([128, 2 * S], BF16, tag="KT")
        V_sb = big.tile([128, KT * NH * (HD + 1)], BF16, tag="V")
        Vv = V_sb[:].rearrange("p (k h j) -> p k h j", k=KT, h=NH)
        OcatT = big.tile([128, 2 * S], BF16, tag="OT")   # [p, t*2048+s]

        nc.vector.memset(Vv[:, :, :, HD:HD + 1], 1.0)
        ident = consts.tile([128, 128], BF16, tag="ident")
        make_identity(nc, ident[:])

        # warm the ACT exp table at t~0
        warm = smal.tile([1, 2], F32, tag="warm")
        nc.vector.memset(warm[:], 0.0)
        nc.scalar.activation(warm[:], warm[:], AF.Exp)

        # warm the PE HAM clock gate during the initial DMA wait: ~4096
        # cycles of junk transposes releases the 4/8 throttle before the
        # first projection issues.
        tpw = psT.tile([128, 128], BF16, tag="tp", name="tpwarm")
        for _ in range(32):
            nc.tensor.transpose(tpw[:], ident[:], ident[:])

        # ---- all input DMAs up front, priority-ordered for the serial
        # HBM pipe (wq/xq first so Q proj starts ASAP; xk/xv halves
        # interleaved to meet scores/PV deadlines; xq2/3 last) ----
        xblk = {}   # ("q"|"k"|"v", chunk) -> tile; halves DMA'd separately

        def x_dma_half(src, kind, c, half):
            if (kind, c) not in xblk:
                xblk[(kind, c)] = xp.tile([128, ET, 512], BF16, tag="xblk",
                                          name=f"x{kind}{c}")
            xb = xblk[(kind, c)]
            sl = slice(c * 512 + half * 256, c * 512 + (half + 1) * 256)
            nc.sync.dma_start(
                xb[:, :, half * 256:(half + 1) * 256],
                src[:, sl].rearrange("(t p) s -> p t s", p=128))

        def x_dma_full(src, kind, c):
            xblk[(kind, c)] = xp.tile([128, ET, 512], BF16, tag="xblk",
                                      name=f"x{kind}{c}")
            nc.sync.dma_start(
                xblk[(kind, c)][:],
                src[:, c * 512:(c + 1) * 512].rearrange("(t p) s -> p t s", p=128))

        nc.sync.dma_start(wq_sb[:], wq.rearrange("(t p) d -> p t d", p=128))
        nc.sync.dma_start(b3_sb[:], b3[:])
        x_dma_full(xqT, "q", 0)
        x_dma_full(xqT, "q", 1)
        nc.sync.dma_start(wk_sb[:], wk.rearrange("(t p) d -> p t d", p=128))
        x_dma_half(xkT, "k", 0, 0); x_dma_half(xkT, "k", 0, 1)
        x_dma_half(xkT, "k", 1, 0); x_dma_half(xkT, "k", 1, 1)
        x_dma_half(xkT, "k", 2, 0); x_dma_half(xkT, "k", 2, 1)
        x_dma_half(xkT, "k", 3, 0); x_dma_half(xkT, "k", 3, 1)
        nc.sync.dma_start(wv_sb[:], wv.rearrange("(t p) d -> p t d", p=128))
        x_dma_half(xvT, "v", 0, 0); x_dma_half(xvT, "v", 0, 1)
        x_dma_half(xvT, "v", 1, 0); x_dma_half(xvT, "v", 1, 1)
        x_dma_half(xvT, "v", 2, 0); x_dma_half(xvT, "v", 2, 1)
        x_dma_half(xvT, "v", 3, 0); x_dma_half(xvT, "v", 3, 1)
        nc.sync.dma_start(wo_sb[:], wo.rearrange("(t p) n -> p t n", p=128))
        x_dma_full(xqT, "q", 2)
        x_dma_full(xqT, "q", 3)

        # ---- filler units: micro-chunked projections / outproj ----
        # Each unit owns one psM (or psT) bank while its chunks are
        # interleaved between tasks; a chunk is ~1024 PE cycles.

        class Unit:
            def __init__(self, key, nchunks, start_fn, chunk_fn, drain_fn):
                self.key = key
                self.n = nchunks
                self.i = 0
                self.start_fn = start_fn
                self.chunk_fn = chunk_fn
                self.drain_fn = drain_fn
                self.state = None

            def emit_chunk(self):
                if self.i == 0:
                    self.state = self.start_fn()
                self.chunk_fn(self.state, self.i)
                self.i += 1
                if self.i == self.n:
                    self.drain_fn(self.state)
                    return True
                return False

        def q_unit(qc, dc):
            # QT_sb[:, dc*S + qc*512 : +512] = (x_qc @ Wq[:, dc])^T + bq
            def start():
                return psM.tile([128, 512], F32, tag="m", name=f"q{qc}{dc}")

            def chunk(ps, i):
                for t in (2 * i, 2 * i + 1):
                    nc.tensor.matmul(
                        ps[:], wq_sb[:, t, dc * 128:(dc + 1) * 128],
                        xblk[("q", qc)][:, t, :],
                        start=(t == 0), stop=(t == ET - 1))

            def drain(ps):
                nc.vector.tensor_scalar_add(
                    QT_sb[:, dc * S + qc * 512: dc * S + (qc + 1) * 512],
                    ps[:], b3_sb[:, dc:dc + 1])
            return Unit(("q", qc, dc), 4, start, chunk, drain)

        def k_unit(g, dc, kt):
            # KT_sb[:, dc*S + kt*128 : +128]; kt in [4g, 4g+4)
            ko = (kt - 4 * g) * 128

            def start():
                return psM.tile([128, 128], F32, tag="m", name=f"k{kt}{dc}")

            def chunk(ps, i):
                for t in range(ET):
                    nc.tensor.matmul(
                        ps[:], wk_sb[:, t, dc * 128:(dc + 1) * 128],
                        xblk[("k", g)][:, t, ko:ko + 128],
                        start=(t == 0), stop=(t == ET - 1))

            def drain(ps):
                nc.vector.tensor_scalar_add(
                    KT_sb[:, dc * S + kt * 128: dc * S + kt * 128 + 128],
                    ps[:], b3_sb[:, 2 + dc:3 + dc])
            return Unit(("k", kt, dc), 1, start, chunk, drain)

        def v_unit(sc, half):
            # k-positions [(sc*4+2*half)*128, +256) of V  (kt pair)
            def start():
                return psM.tile([128, 2, HC], F32, tag="m",
                                name=f"v{sc}{half}")

            def chunk(psv, i):
                stl, tb = divmod(i, 2)
                st = 2 * half + stl
                for t in (4 * tb, 4 * tb + 1, 4 * tb + 2, 4 * tb + 3):
                    nc.tensor.matmul(
                        psv[:, stl],
                        xblk[("v", sc)][:, t, st * 128:(st + 1) * 128],
                        wv_sb[:, t, :],
                        start=(t == 0 and stl == 0),
                        stop=(t == ET - 1 and stl == 1))

            def drain(psv):
                nc.vector.tensor_copy(
                    Vv[:, sc * 4 + 2 * half: sc * 4 + 2 * half + 2, :, 0:HD],
                    psv[:].rearrange("p s (h j) -> p s h j", h=NH))
            return Unit(("v", sc, half), 4, start, chunk, drain)

        op_alt = [0]

        def o_unit(qh, ec, jq, tail=False):
            # outT[ec*128:+128, qh*1024 + jq*512 : +512]
            qc = 2 * qh + jq

            def start():
                alt = op_alt[0]
                op_alt[0] += 1
                use_t = tail and alt % 2
                pool, tag = (psT, "tp") if use_t else (psM, "m")
                return (pool.tile([128, 512], F32, tag=tag, name=f"o{ec}{qc}"),
                        use_t)

            def chunk(st, i):
                po, _ = st
                for t in range(2):
                    nc.tensor.matmul(
                        po[:], wo_sb[:, t, ec * 128:(ec + 1) * 128],
                        OcatT[:, t * S + qc * 512: t * S + (qc + 1) * 512],
                        start=(t == 0), stop=(t == 1))

            def drain(st):
                po, use_t = st
                stg = outp.tile([128, 512], BF16, tag="stg", name=f"s{ec}{qc}")
                if use_t:
                    nc.scalar.activation(stg[:], po[:], AF.Identity)
                else:
                    nc.vector.tensor_copy(stg[:], po[:])
                nc.sync.dma_start(
                    outT[ec * 128:(ec + 1) * 128,
                         qh * 1024 + jq * 512: qh * 1024 + (jq + 1) * 512],
                    stg[:])
            return Unit(("o", qh, ec, jq), 1, start, chunk, drain)

        # filler queue in pacing-priority order; hard deadlines are
        # enforced by require() before each consumer, which lets a unit
        # jump the queue.
        fillers = []
        fillers += [q_unit(0, 0), q_unit(1, 0)]          # ramp, pre-required
        fillers += [k_unit(0, 0, kt) for kt in range(0, 4)]
        fillers += [q_unit(0, 1), q_unit(1, 1)]
        fillers += [k_unit(1, 0, kt) for kt in range(4, 8)]
        fillers += [k_unit(2, 0, kt) for kt in range(8, 12)]
        fillers += [k_unit(3, 0, kt) for kt in range(12, 16)]
        fillers += [v_unit(0, 0), v_unit(0, 1)]
        fillers += [v_unit(1, 0), v_unit(1, 1)]
        fillers += [k_unit(0, 1, kt) for kt in range(0, 4)]
        fillers += [k_unit(1, 1, kt) for kt in range(4, 8)]
        fillers += [v_unit(2, 0), v_unit(2, 1)]
        fillers += [v_unit(3, 0), v_unit(3, 1)]
        fillers += [k_unit(2, 1, kt) for kt in range(8, 12)]
        fillers += [k_unit(3, 1, kt) for kt in range(12, 16)]
        fillers += [q_unit(2, 0), q_unit(2, 1), q_unit(3, 0), q_unit(3, 1)]

        done_units = set()
        unit_by_key = {u.key: u for u in fillers}

        def emit_next_filler():
            while fillers:
                u = fillers[0]
                if u.key in done_units:
                    fillers.pop(0)
                    continue
                if u.emit_chunk():
                    done_units.add(u.key)
                    fillers.pop(0)
                return True
            return False

        def require(key):
            u = unit_by_key.get(key)
            if u is None or key in done_units:
                return
            # psM is single-tenant: finish any mid-flight unit before
            # this one takes the bank, or their accumulations would alias
            if (fillers and fillers[0] is not u
                    and fillers[0].key not in done_units
                    and 0 < fillers[0].i < fillers[0].n):
                head = fillers.pop(0)
                while not head.emit_chunk():
                    pass
                done_units.add(head.key)
            while key not in done_units:
                if u.emit_chunk():
                    done_units.add(key)
                    if fillers and fillers[0] is u:
                        fillers.pop(0)

        # ---- attention task machinery ----
        pts = {}
        psos = {}

        def attn_s(h, qh, kt):
            require(("q", 2 * qh, h // 2))
            require(("q", 2 * qh + 1, h // 2))
            require(("k", kt, h // 2))
            dch, po = h // 2, 64 * (h % 2)
            pss = psS.tile([128, 1024], F32, tag="s", name=f"pss{h}{qh}{kt}")
            for j in range(2):
                qoff = dch * S + (qh * 2 + j) * 512
                nc.tensor.matmul(
                    pss[:, j * 512:(j + 1) * 512],
                    KT_sb[po:po + 64, dch * S + kt * 128: dch * S + (kt + 1) * 128],
                    QT_sb[po:po + 64, qoff:qoff + 512],
                    start=True, stop=True)
            pt = ptp.tile([128, 1024], BF16, tag="pt", name=f"pt{h}{qh}{kt}")
            nc.scalar.activation(pt[:], pss[:], AF.Exp, scale=SCALE)
            pts[(h, qh, kt)] = pt

        def attn_pv(h, qh, kt):
            require(("v", kt // 4, (kt % 4) // 2))
            if kt == 0:
                psos[(h, qh)] = psV.tile([128, 1024], F32, tag="v",
                                         name=f"pso{h}{qh}")
            pso = psos[(h, qh)]
            pt = pts.pop((h, qh, kt))
            for qt in range(8):
                nc.tensor.matmul(
                    pso[:, qt * 128: qt * 128 + HD + 1],
                    pt[:, qt * 128:(qt + 1) * 128],
                    Vv[:, kt, h, 0:HD + 1],
                    start=(kt == 0 and qt % 4 == 0),
                    stop=(kt == KT - 1 and qt % 4 == 3))

        def attn_norm(h, qh):
            dch, po = h // 2, 64 * (h % 2)
            pso = psos.pop((h, qh))
            psv8 = pso[:].rearrange("p (t c) -> p t c", t=8)
            recip = smal.tile([128, 8, 1], F32, tag="recip", name=f"rc{h}{qh}")
            nc.vector.reciprocal(recip[:], psv8[:, :, HD:HD + 1])
            tmpb = tmb.tile([128, 8, HD], BF16, tag="tmp", name=f"tm{h}{qh}")
            for qt in range(8):
                nc.vector.tensor_scalar_mul(
                    tmpb[:, qt], psv8[:, qt, 0:HD], recip[:, qt, 0:1])
            tp = psT.tile([HD, 1024], BF16, tag="tp", name=f"tp{h}{qh}")
            for qt in range(8):
                nc.tensor.transpose(
                    tp[:, qt * 128:(qt + 1) * 128], tmpb[:, qt], ident[:])
            nc.vector.tensor_copy(
                OcatT[po:po + 64, dch * S + qh * 1024: dch * S + (qh + 1) * 1024],
                tp[:])

        normed = set()

        def run_pv(task):
            h, qh, kt = task
            attn_pv(h, qh, kt)
            if kt == KT - 1:
                attn_norm(h, qh)
                normed.add((h, qh))

        def v_emitted(task):
            h, qh, kt = task
            return ("v", kt // 4, (kt % 4) // 2) in done_units

        # ---- task stream ----
        tasks = [(h, 0, kt) for h in range(NH) for kt in range(KT)]
        tasks += [(h, 1, kt) for h in range(NH) for kt in range(KT)]

        oqueue = []  # outproj(qh=0) units, released under the qh1 era
        oq_released = False
        pv_pending = []
        require(("q", 0, 0))
        require(("q", 1, 0))
        for i, task in enumerate(tasks):
            attn_s(*task)
            pv_pending.append(task)
            # drain PVs LAG behind; during the ramp defer (up to the pt
            # backlog cap) while their V tiles are still being projected.
            # The final phase runs at lag 1 so the last norm (which gates
            # the whole output-projection tail) lands right after its exp.
            lag = 1 if i >= len(tasks) - KT else LAG
            while pv_pending and (
                    len(pv_pending) > PTS_CAP
                    or (len(pv_pending) > lag and v_emitted(pv_pending[0]))):
                run_pv(pv_pending.pop(0))
            # one filler chunk per task (7 of 8) keeps PE fed without
            # starving ACT of score tiles
            if i % 8 != 7:
                if not emit_next_filler() and oqueue:
                    oqueue.pop(0).emit_chunk()
            # release outproj(0) once all qh0 phases are normed
            if not oq_released and all((h, 0) in normed for h in range(NH)):
                oq_released = True
                oqueue += [o_unit(0, ec, jq) for ec in range(ET)
                           for jq in range(2)]
        while pv_pending:
            run_pv(pv_pending.pop(0))
        while emit_next_filler():
            pass
        while oqueue:
            oqueue.pop(0).emit_chunk()
        # tail: outproj(qh=1) in 16 [128,512] chunks rotating through 5
        # PSUM slot-groups (scores + pv + transpose + misc banks, all free
        # after the last exp/norm) so drains (alternating DVE / ACT) never
        # block the matmul stream
        tail_pools = [(psS, "s"), (psV, "v"), (psS, "s"), (psT, "tp"),
                      (psM, "m")]
        u = 0
        for ec in range(ET):
            stg = outp.tile([128, 1024], BF16, tag="stg2", name=f"ts{ec}")
            for jq, qc in enumerate((2, 3)):
                pool, tag = tail_pools[u % len(tail_pools)]
                po = pool.tile([128, 512], F32, tag=tag, name=f"tpo{ec}{jq}")
                for t in range(2):
                    nc.tensor.matmul(
                        po[:], wo_sb[:, t, ec * 128:(ec + 1) * 128],
                        OcatT[:, t * S + qc * 512: t * S + (qc + 1) * 512],
                        start=(t == 0), stop=(t == 1))
                if u % 2:
                    nc.scalar.activation(stg[:, jq * 512:(jq + 1) * 512],
                                         po[:], AF.Identity)
                else:
                    nc.vector.tensor_copy(stg[:, jq * 512:(jq + 1) * 512],
                                          po[:])
                u += 1
            nc.sync.dma_start(
                outT[ec * 128:(ec + 1) * 128, 1024:2048], stg[:])


def build_program(stop_after=None, reps=1):
    nc = bacc.Bacc("TRN2", target_bir_lowering=False, debug=False, num_devices=8)
    io = {
        "xqT": nc.dram_tensor("xqT", [E, S], BF16, kind="ExternalInput").ap(),
        "xkT": nc.dram_tensor("xkT", [E, S], BF16, kind="ExternalInput").ap(),
        "xvT": nc.dram_tensor("xvT", [E, S], BF16, kind="ExternalInput").ap(),
        "wq": nc.dram_tensor("wq", [E, HC], BF16, kind="ExternalInput").ap(),
        "wk": nc.dram_tensor("wk", [E, HC], BF16, kind="ExternalInput").ap(),
        "wv": nc.dram_tensor("wv", [E, HC], BF16, kind="ExternalInput").ap(),
        "wo": nc.dram_tensor("wo", [HC, E], BF16, kind="ExternalInput").ap(),
        "b3": nc.dram_tensor("b3", [128, 4], F32, kind="ExternalInput").ap(),
        "outT": nc.dram_tensor("outT", [E, S], BF16, kind="ExternalOutput").ap(),
    }
    with tile.TileContext(nc) as tc:
        for _ in range(reps):
            _emit(nc, tc, io)
    nc.compile()
    return nc


def make_in_maps(q, k, v, Wq, bq, Wk, bk, Wv, bv, Wo, bo):
    q, k, v = (np.asarray(x, np.float32) for x in (q, k, v))
    Wq, Wk, Wv, Wo = (np.asarray(x, np.float32) for x in (Wq, Wk, Wv, Wo))
    bq, bk, bv = (np.asarray(x, np.float32) for x in (bq, bk, bv))
    xT = {b: {n: np.ascontiguousarray(a[b].T).astype(NPBF16)
              for n, a in (("xqT", q), ("xkT", k), ("xvT", v))}
          for b in range(2)}
    in_maps = []
    for c in range(8):
        b, g = divmod(c, 4)
        cs = slice(g * HC, (g + 1) * HC)
        b3 = np.stack([bq[cs].reshape(2, 128)[0], bq[cs].reshape(2, 128)[1],
                       bk[cs].reshape(2, 128)[0], bk[cs].reshape(2, 128)[1]],
                      axis=1)
        in_maps.append({
            "xqT": xT[b]["xqT"], "xkT": xT[b]["xkT"], "xvT": xT[b]["xvT"],
            "wq": np.ascontiguousarray(Wq[:, cs]).astype(NPBF16),
            "wk": np.ascontiguousarray(Wk[:, cs]).astype(NPBF16),
            "wv": np.ascontiguousarray(Wv[:, cs]).astype(NPBF16),
            "wo": np.ascontiguousarray(Wo[cs, :]).astype(NPBF16),
            "b3": np.ascontiguousarray(b3, np.float32),
        })
    return in_maps


def kernel(q, k, v, Wq, bq, Wk, bk, Wv, bv, Wo, bo):
    if "nc" not in _CACHE:
        _CACHE["nc"] = build_program()
    nc = _CACHE["nc"]
    in_maps = make_in_maps(q, k, v, Wq, bq, Wk, bk, Wv, bv, Wo, bo)
    res = run_bass_kernel_spmd(nc, in_maps, list(range(8))).results
    bo = np.asarray(bo, np.float32)
    bv = np.asarray(bv, np.float32)
    Wo = np.asarray(Wo, np.float32)
    extra = bv @ Wo + bo  # bv folds through the output projection
    out = np.empty((2, S, E), np.float32)
    for b in range(2):
        acc = res[4 * b]["outT"].astype(np.float32)
        for g in range(1, 4):
            acc += res[4 * b + g]["outT"].astype(np.float32)
        out[b] = acc.T + extra
    return out


# revision 41
# speedup vs baseline: 1.2542x; 1.0590x over previous
"""MultiHeadAttention kernel for 8 TRN2 NeuronCores (Bass/Tile, SPMD).

Sharding: core c = (batch b, head-group g) with b = c//4, g = c%4.
Each core computes heads [4g, 4g+4) of batch b:
  Q/K/V column-parallel projections (256 hidden cols per core),
  full attention over its 4 heads (S=2048, head_dim=64),
  row-parallel output projection -> partial [S, EMB] output.
Host gathers: out[b] = sum_g partial[b,g] + bo  (the TP all-reduce).

Storage dtype is bf16; accumulation in fp32 PSUM. Softmax skips the
max-subtraction (scores are O(1)); the denominator comes free from a
ones-column appended to V in the PV matmul.

Schedule (v2): the ACT engine (exp over all S^2 scores) is the binding
resource (~133us busy) with PE nearly equal (~140us). All input DMAs
are issued up front in one priority-ordered stream on the serial
~360GB/s HBM pipe. Projections and the output projection are chopped
into ~1024-cycle micro-chunks interleaved between (scores, exp, PV)
tasks so the in-order PE stream never starves ACT for more than the
one-tile PSUM lookahead. PSUM: scores double-buffer 4 banks, PV
accumulator 2, transpose 1, proj/outproj scratch 1.
"""

from contextlib import ExitStack

import ml_dtypes
import numpy as np

import concourse.bass as bass
import concourse.mybir as mybir
import concourse.tile as tile
from concourse.masks import make_identity
from concourse import bacc
from concourse.bass_utils import run_bass_kernel_spmd

F32 = mybir.dt.float32
BF16 = mybir.dt.bfloat16
F8 = mybir.dt.float8e4
AF = mybir.ActivationFunctionType
NPBF16 = np.dtype(ml_dtypes.bfloat16)
NPF8 = np.dtype(ml_dtypes.float8_e4m3)

S = 2048          # sequence length
E = 1024          # embedding dim
HC = 256          # hidden cols per core (= 4 heads * 64)
NH = 4            # heads per core
HD = 64           # head dim
ET = E // 128     # e-tiles (8)
KT = S // 128     # 128-wide k tiles (16)
SCALE = 1.0 / 8.0  # 1/sqrt(HD)

LAG = 4           # PV trails the scores/exp stream by this many tasks
PTS_CAP = 20      # max outstanding exp tiles awaiting PV

_CACHE: dict = {}


def _emit(nc, tc, io):
    xqT, xkT, xvT = io["xqT"], io["xkT"], io["xvT"]
    wq, wk, wv, wo = io["wq"], io["wk"], io["wv"], io["wo"]
    b3 = io["b3"]
    outT = io["outT"]

    with ExitStack() as ctx:
        consts = ctx.enter_context(tc.tile_pool(name="consts", bufs=1))
        big = ctx.enter_context(tc.tile_pool(name="big", bufs=1))
        xp = ctx.enter_context(tc.tile_pool(name="xp", bufs=10))
        ptp = ctx.enter_context(tc.tile_pool(name="pt", bufs=PTS_CAP + 3))
        smal = ctx.enter_context(tc.tile_pool(name="small", bufs=3))
        tmb = ctx.enter_context(tc.tile_pool(name="tmb", bufs=2))
        outp = ctx.enter_context(tc.tile_pool(name="outstage", bufs=4))
        psS = ctx.enter_context(tc.tile_pool(name="psS", bufs=2, space="PSUM"))
        psV = ctx.enter_context(tc.tile_pool(name="psV", bufs=1, space="PSUM"))
        psT = ctx.enter_context(tc.tile_pool(name="psT", bufs=1, space="PSUM"))
        psM = ctx.enter_context(tc.tile_pool(name="psM", bufs=1, space="PSUM"))

        # ---- SBUF destinations ----
        wq_sb = consts.tile([128, ET, HC], BF16, tag="wq")   # [p, t, d]
        wk_sb = consts.tile([128, ET, HC], BF16, tag="wk")
        wv_sb = consts.tile([128, ET, HC], BF16, tag="wv")
        wo_sb = consts.tile([128, 2, E], BF16, tag="wo")     # [p, t, n]
        b3_sb = consts.tile([128, 4], F32, tag="b3")  # bq|bk, 2 cols each

        QT_sb = big.tile([128, 2 * S], BF16, tag="QT")   # [p, dc*2048+s]
        KT_sb = big.tile([128, 2 * S], BF16, tag="KT")
        V_sb = big.tile([128, KT * NH * (HD + 1)], BF16, tag="V")
        Vv = V_sb[:].rearrange("p (k h j) -> p k h j", k=KT, h=NH)
        OcatT = big.tile([128, 2 * S], BF16, tag="OT")   # [p, t*2048+s]

        nc.vector.memset(Vv[:, :, :, HD:HD + 1], 1.0)
        ident = consts.tile([128, 128], BF16, tag="ident")
        make_identity(nc, ident[:])

        # warm the ACT exp table at t~0
        warm = smal.tile([1, 2], F32, tag="warm")
        nc.vector.memset(warm[:], 0.0)
        nc.scalar.activation(warm[:], warm[:], AF.Exp)

        # warm the PE HAM clock gate during the initial DMA wait: ~4096
        # cycles of junk transposes releases the 4/8 throttle before the
        # first projection issues.
        tpw = psT.tile([128, 128], BF16, tag="tp", name="tpwarm")
        for _ in range(32):
            nc.tensor.transpose(tpw[:], ident[:], ident[:])

        # ---- all input DMAs up front, priority-ordered for the serial
        # HBM pipe (wq/xq first so Q proj starts ASAP; xk/xv halves
        # interleaved to meet scores/PV deadlines; xq2/3 last) ----
        xblk = {}   # ("q"|"k"|"v", chunk) -> tile; halves DMA'd separately

        def x_dma_half(src, kind, c, half):
            if (kind, c) not in xblk:
                xblk[(kind, c)] = xp.tile([128, ET, 512], BF16, tag="xblk",
                                          name=f"x{kind}{c}")
            xb = xblk[(kind, c)]
            sl = slice(c * 512 + half * 256, c * 512 + (half + 1) * 256)
            nc.sync.dma_start(
                xb[:, :, half * 256:(half + 1) * 256],
                src[:, sl].rearrange("(t p) s -> p t s", p=128))

        def x_dma_full(src, kind, c):
            xblk[(kind, c)] = xp.tile([128, ET, 512], BF16, tag="xblk",
                                      name=f"x{kind}{c}")
            nc.sync.dma_start(
                xblk[(kind, c)][:],
                src[:, c * 512:(c + 1) * 512].rearrange("(t p) s -> p t s", p=128))

        def x_dma_quarter(src, kind, c, qtr):
            if (kind, c) not in xblk:
                xblk[(kind, c)] = xp.tile([128, ET, 512], BF16, tag="xblk",
                                          name=f"x{kind}{c}")
            xb = xblk[(kind, c)]
            sl = slice(c * 512 + qtr * 128, c * 512 + (qtr + 1) * 128)
            nc.sync.dma_start(
                xb[:, :, qtr * 128:(qtr + 1) * 128],
                src[:, sl].rearrange("(t p) s -> p t s", p=128))

        nc.sync.dma_start(wq_sb[:], wq.rearrange("(t p) d -> p t d", p=128))
        nc.sync.dma_start(b3_sb[:], b3[:])
        x_dma_full(xqT, "q", 0)
        x_dma_full(xqT, "q", 1)
        nc.sync.dma_start(wk_sb[:], wk.rearrange("(t p) d -> p t d", p=128))
        x_dma_half(xkT, "k", 0, 0); x_dma_half(xkT, "k", 0, 1)
        x_dma_half(xkT, "k", 1, 0); x_dma_half(xkT, "k", 1, 1)
        x_dma_half(xkT, "k", 2, 0); x_dma_half(xkT, "k", 2, 1)
        x_dma_half(xkT, "k", 3, 0); x_dma_half(xkT, "k", 3, 1)
        nc.sync.dma_start(wv_sb[:], wv.rearrange("(t p) d -> p t d", p=128))
        x_dma_half(xvT, "v", 0, 0); x_dma_half(xvT, "v", 0, 1)
        x_dma_half(xvT, "v", 1, 0); x_dma_half(xvT, "v", 1, 1)
        x_dma_half(xvT, "v", 2, 0); x_dma_half(xvT, "v", 2, 1)
        x_dma_half(xvT, "v", 3, 0); x_dma_half(xvT, "v", 3, 1)
        nc.sync.dma_start(wo_sb[:], wo.rearrange("(t p) n -> p t n", p=128))
        x_dma_full(xqT, "q", 2)
        x_dma_full(xqT, "q", 3)

        # ---- filler units: micro-chunked projections / outproj ----
        # Each unit owns one psM (or psT) bank while its chunks are
        # interleaved between tasks; a chunk is ~1024 PE cycles.

        class Unit:
            def __init__(self, key, nchunks, start_fn, chunk_fn, drain_fn):
                self.key = key
                self.min_task = 0   # earliest task whose pacing may emit this
                self.n = nchunks
                self.i = 0
                self.start_fn = start_fn
                self.chunk_fn = chunk_fn
                self.drain_fn = drain_fn
                self.state = None

            def emit_chunk(self):
                if self.i == 0:
                    self.state = self.start_fn()
                self.chunk_fn(self.state, self.i)
                self.i += 1
                if self.i == self.n:
                    self.drain_fn(self.state)
                    return True
                return False

        def q_unit(qc, dc):
            # QT_sb[:, dc*S + qc*512 : +512] = (x_qc @ Wq[:, dc])^T + bq
            def start():
                return psM.tile([128, 512], F32, tag="m", name=f"q{qc}{dc}")

            def chunk(ps, i):
                for t in (2 * i, 2 * i + 1):
                    nc.tensor.matmul(
                        ps[:], wq_sb[:, t, dc * 128:(dc + 1) * 128],
                        xblk[("q", qc)][:, t, :],
                        start=(t == 0), stop=(t == ET - 1))

            def drain(ps):
                nc.vector.tensor_scalar_add(
                    QT_sb[:, dc * S + qc * 512: dc * S + (qc + 1) * 512],
                    ps[:], b3_sb[:, dc:dc + 1])
            return Unit(("q", qc, dc), 4, start, chunk, drain)

        def k_unit(g, dc, kt):
            # KT_sb[:, dc*S + kt*128 : +128]; kt in [4g, 4g+4)
            ko = (kt - 4 * g) * 128

            def start():
                return psM.tile([128, 128], F32, tag="m", name=f"k{kt}{dc}")

            def chunk(ps, i):
                for t in range(ET):
                    nc.tensor.matmul(
                        ps[:], wk_sb[:, t, dc * 128:(dc + 1) * 128],
                        xblk[("k", g)][:, t, ko:ko + 128],
                        start=(t == 0), stop=(t == ET - 1))

            def drain(ps):
                nc.vector.tensor_scalar_add(
                    KT_sb[:, dc * S + kt * 128: dc * S + kt * 128 + 128],
                    ps[:], b3_sb[:, 2 + dc:3 + dc])
            return Unit(("k", kt, dc), 1, start, chunk, drain)

        def v_unit(sc, half):
            # k-positions [(sc*4+2*half)*128, +256) of V  (kt pair)
            def start():
                return psM.tile([128, 2, HC], F32, tag="m",
                                name=f"v{sc}{half}")

            def chunk(psv, i):
                stl, tb = divmod(i, 2)
                st = 2 * half + stl
                for t in (4 * tb, 4 * tb + 1, 4 * tb + 2, 4 * tb + 3):
                    nc.tensor.matmul(
                        psv[:, stl],
                        xblk[("v", sc)][:, t, st * 128:(st + 1) * 128],
                        wv_sb[:, t, :],
                        start=(t == 0 and stl == 0),
                        stop=(t == ET - 1 and stl == 1))

            def drain(psv):
                nc.vector.tensor_copy(
                    Vv[:, sc * 4 + 2 * half: sc * 4 + 2 * half + 2, :, 0:HD],
                    psv[:].rearrange("p s (h j) -> p s h j", h=NH))
            return Unit(("v", sc, half), 4, start, chunk, drain)

        op_alt = [0]

        def o_unit(qh, ec, jq, tail=False):
            # outT[ec*128:+128, qh*1024 + jq*512 : +512]
            qc = 2 * qh + jq

            def start():
                alt = op_alt[0]
                op_alt[0] += 1
                use_t = tail and alt % 2
                pool, tag = (psT, "tp") if use_t else (psM, "m")
                return (pool.tile([128, 512], F32, tag=tag, name=f"o{ec}{qc}"),
                        use_t)

            def chunk(st, i):
                po, _ = st
                for t in range(2):
                    nc.tensor.matmul(
                        po[:], wo_sb[:, t, ec * 128:(ec + 1) * 128],
                        OcatT[:, t * S + qc * 512: t * S + (qc + 1) * 512],
                        start=(t == 0), stop=(t == 1))

            def drain(st):
                po, use_t = st
                stg = outp.tile([128, 512], BF16, tag="stg", name=f"s{ec}{qc}")
                if use_t:
                    nc.scalar.activation(stg[:], po[:], AF.Identity)
                else:
                    nc.vector.tensor_copy(stg[:], po[:])
                nc.sync.dma_start(
                    outT[ec * 128:(ec + 1) * 128,
                         qh * 1024 + jq * 512: qh * 1024 + (jq + 1) * 512],
                    stg[:])
            return Unit(("o", qh, ec, jq), 1, start, chunk, drain)

        # filler queue in pacing-priority order; hard deadlines are
        # enforced by require() before each consumer, which lets a unit
        # jump the queue.
        # Queue ordered by consumer deadline; min_task gates each unit on
        # its DMA's arrival (pipe position) so pacing never parks the
        # in-order PE stream on an un-landed transfer. require() still
        # force-finishes any unit whose consumer arrives early.
        def _mt(u, t):
            u.min_task = t
            return u

        fillers = []
        fillers += [q_unit(0, 0), q_unit(1, 0)]          # ramp, pre-required
        fillers += [k_unit(0, 0, kt) for kt in range(0, 4)]
        fillers += [_mt(k_unit(1, 0, kt), 3) for kt in range(4, 8)]
        fillers += [_mt(k_unit(2, 0, kt), 6) for kt in range(8, 12)]
        fillers += [_mt(k_unit(3, 0, kt), 8) for kt in range(12, 16)]
        fillers += [_mt(v_unit(0, 0), 13), _mt(v_unit(0, 1), 14)]
        fillers += [_mt(v_unit(1, 0), 16), _mt(v_unit(1, 1), 17)]
        fillers += [_mt(v_unit(2, 0), 19), _mt(v_unit(2, 1), 20)]
        fillers += [q_unit(0, 1), q_unit(1, 1)]
        fillers += [_mt(k_unit(0, 1, kt), 1) for kt in range(0, 4)]
        fillers += [_mt(v_unit(3, 0), 22), _mt(v_unit(3, 1), 23)]
        fillers += [_mt(k_unit(1, 1, kt), 3) for kt in range(4, 8)]
        fillers += [_mt(k_unit(2, 1, kt), 6) for kt in range(8, 12)]
        fillers += [_mt(k_unit(3, 1, kt), 8) for kt in range(12, 16)]
        fillers += [_mt(q_unit(2, 0), 26), _mt(q_unit(3, 0), 29)]
        fillers += [_mt(q_unit(2, 1), 26), _mt(q_unit(3, 1), 29)]

        done_units = set()
        unit_by_key = {u.key: u for u in fillers}

        def emit_next_filler(i=1 << 30):
            while fillers and fillers[0].key in done_units:
                fillers.pop(0)
            if not fillers:
                return False
            u = fillers[0]
            if u.i == 0 and u.min_task > i:
                # head's DMA not landed yet: scan for a ready unit and
                # move it to the head (psM stays single-tenant)
                for j in range(1, len(fillers)):
                    w = fillers[j]
                    if (w.key not in done_units and w.i == 0
                            and w.min_task <= i):
                        fillers.pop(j)
                        fillers.insert(0, w)
                        u = w
                        break
                else:
                    return False
            if u.emit_chunk():
                done_units.add(u.key)
                fillers.pop(0)
            return True

        def require(key):
            u = unit_by_key.get(key)
            if u is None or key in done_units:
                return
            # psM is single-tenant: finish any mid-flight unit before
            # this one takes the bank, or their accumulations would alias
            if (fillers and fillers[0] is not u
                    and fillers[0].key not in done_units
                    and 0 < fillers[0].i < fillers[0].n):
                head = fillers.pop(0)
                while not head.emit_chunk():
                    pass
                done_units.add(head.key)
            while key not in done_units:
                if u.emit_chunk():
                    done_units.add(key)
                    if fillers and fillers[0] is u:
                        fillers.pop(0)

        # ---- attention task machinery ----
        pts = {}
        psos = {}

        def attn_s(h, qh, kt):
            require(("q", 2 * qh, h // 2))
            require(("q", 2 * qh + 1, h // 2))
            require(("k", kt, h // 2))
            dch, po = h // 2, 64 * (h % 2)
            pss = psS.tile([128, 1024], F32, tag="s", name=f"pss{h}{qh}{kt}")
            for j in range(2):
                qoff = dch * S + (qh * 2 + j) * 512
                nc.tensor.matmul(
                    pss[:, j * 512:(j + 1) * 512],
                    KT_sb[po:po + 64, dch * S + kt * 128: dch * S + (kt + 1) * 128],
                    QT_sb[po:po + 64, qoff:qoff + 512],
                    start=True, stop=True)
            pt = ptp.tile([128, 1024], BF16, tag="pt", name=f"pt{h}{qh}{kt}")
            nc.scalar.activation(pt[:], pss[:], AF.Exp, scale=SCALE)
            pts[(h, qh, kt)] = pt

        def attn_pv(h, qh, kt):
            require(("v", kt // 4, (kt % 4) // 2))
            if kt == 0:
                psos[(h, qh)] = psV.tile([128, 1024], F32, tag="v",
                                         name=f"pso{h}{qh}")
            pso = psos[(h, qh)]
            pt = pts.pop((h, qh, kt))
            for qt in range(8):
                nc.tensor.matmul(
                    pso[:, qt * 128: qt * 128 + HD + 1],
                    pt[:, qt * 128:(qt + 1) * 128],
                    Vv[:, kt, h, 0:HD + 1],
                    start=(kt == 0 and qt % 4 == 0),
                    stop=(kt == KT - 1 and qt % 4 == 3))

        def attn_norm(h, qh):
            dch, po = h // 2, 64 * (h % 2)
            pso = psos.pop((h, qh))
            # one copy PSUM->SBUF frees the single pso buffer for the next
            # phase's PV after ~0.7us instead of holding it through the
            # whole recip+mul chain
            stage = tmb.tile([128, 8, 128], F32, tag="stage", name=f"sg{h}{qh}")
            nc.vector.tensor_copy(
                stage[:], pso[:].rearrange("p (t c) -> p t c", t=8))
            psv8 = stage
            recip = smal.tile([128, 8, 1], F32, tag="recip", name=f"rc{h}{qh}")
            nc.vector.reciprocal(recip[:], psv8[:, :, HD:HD + 1])
            tmpb = tmb.tile([128, 8, HD], BF16, tag="tmp", name=f"tm{h}{qh}")
            for qt in range(8):
                nc.vector.tensor_scalar_mul(
                    tmpb[:, qt], psv8[:, qt, 0:HD], recip[:, qt, 0:1])
            tp = psT.tile([HD, 1024], BF16, tag="tp", name=f"tp{h}{qh}")
            for qt in range(8):
                nc.tensor.transpose(
                    tp[:, qt * 128:(qt + 1) * 128], tmpb[:, qt], ident[:])
            if (h, qh) == (NH - 1, 1):
                # final phase: split the copy so the output-projection
                # tail's first matmuls start half a copy earlier
                for hf in range(2):
                    nc.vector.tensor_copy(
                        OcatT[po:po + 64,
                              dch * S + qh * 1024 + hf * 512:
                              dch * S + qh * 1024 + (hf + 1) * 512],
                        tp[:, hf * 512:(hf + 1) * 512])
            else:
                nc.vector.tensor_copy(
                    OcatT[po:po + 64,
                          dch * S + qh * 1024: dch * S + (qh + 1) * 1024],
                    tp[:])

        normed = set()

        def run_pv(task):
            h, qh, kt = task
            attn_pv(h, qh, kt)
            if kt == KT - 1:
                attn_norm(h, qh)
                normed.add((h, qh))

        def v_emitted(task):
            h, qh, kt = task
            return ("v", kt // 4, (kt % 4) // 2) in done_units

        # ---- task stream ----
        tasks = [(h, 0, kt) for h in range(NH) for kt in range(KT)]
        tasks += [(h, 1, kt) for h in range(NH) for kt in range(KT)]

        oqueue = []  # outproj(qh=0) units, released under the qh1 era
        oq_released = False
        pv_pending = []
        require(("q", 0, 0))
        require(("q", 1, 0))
        for i, task in enumerate(tasks):
            attn_s(*task)
            pv_pending.append(task)
            # drain PVs LAG behind; during the ramp defer (up to the pt
            # backlog cap) while their V tiles are still being projected.
            # The final phase runs at lag 1 so the last norm (which gates
            # the whole output-projection tail) lands right after its exp.
            lag = 1 if i >= len(tasks) - KT else LAG
            drained = 0
            while pv_pending and drained < 3 and (
                    len(pv_pending) > PTS_CAP
                    or (len(pv_pending) > lag and v_emitted(pv_pending[0]))):
                run_pv(pv_pending.pop(0))
                drained += 1
            # pacing: 2 fillers/task in the DMA-dead early ramp, then 1
            # every task until the queue drains, then 7-of-8 for outproj
            nfill = 2 if i < 8 else 1
            if i >= 48 and i % 8 == 7:
                nfill = 0
            for _ in range(nfill):
                if not emit_next_filler(i) and oqueue:
                    oqueue.pop(0).emit_chunk()
                    break
            # release outproj(0) once all qh0 phases are normed
            if not oq_released and all((h, 0) in normed for h in range(NH)):
                oq_released = True
                oqueue += [o_unit(0, ec, jq) for ec in range(ET)
                           for jq in range(2)]
        while pv_pending:
            run_pv(pv_pending.pop(0))
        while emit_next_filler():
            pass
        while oqueue:
            oqueue.pop(0).emit_chunk()
        # tail: outproj(qh=1) in 16 [128,512] chunks rotating through 5
        # PSUM slot-groups (scores + pv + transpose + misc banks, all free
        # after the last exp/norm) so drains (alternating DVE / ACT) never
        # block the matmul stream
        tail_pools = [(psS, "s"), (psV, "v"), (psS, "s"), (psT, "tp"),
                      (psM, "m")]
        u = 0
        for ec in range(ET):
            stg = outp.tile([128, 1024], BF16, tag="stg2", name=f"ts{ec}")
            for jq, qc in enumerate((2, 3)):
                pool, tag = tail_pools[u % len(tail_pools)]
                po = pool.tile([128, 512], F32, tag=tag, name=f"tpo{ec}{jq}")
                for t in range(2):
                    nc.tensor.matmul(
                        po[:], wo_sb[:, t, ec * 128:(ec + 1) * 128],
                        OcatT[:, t * S + qc * 512: t * S + (qc + 1) * 512],
                        start=(t == 0), stop=(t == 1))
                if u % 2:
                    nc.scalar.activation(stg[:, jq * 512:(jq + 1) * 512],
                                         po[:], AF.Identity)
                else:
                    nc.vector.tensor_copy(stg[:, jq * 512:(jq + 1) * 512],
                                          po[:])
                u += 1
            nc.sync.dma_start(
                outT[ec * 128:(ec + 1) * 128, 1024:2048], stg[:])


def build_program(stop_after=None, reps=1):
    nc = bacc.Bacc("TRN2", target_bir_lowering=False, debug=False, num_devices=8)
    io = {
        "xqT": nc.dram_tensor("xqT", [E, S], BF16, kind="ExternalInput").ap(),
        "xkT": nc.dram_tensor("xkT", [E, S], BF16, kind="ExternalInput").ap(),
        "xvT": nc.dram_tensor("xvT", [E, S], BF16, kind="ExternalInput").ap(),
        "wq": nc.dram_tensor("wq", [E, HC], BF16, kind="ExternalInput").ap(),
        "wk": nc.dram_tensor("wk", [E, HC], BF16, kind="ExternalInput").ap(),
        "wv": nc.dram_tensor("wv", [E, HC], BF16, kind="ExternalInput").ap(),
        "wo": nc.dram_tensor("wo", [HC, E], BF16, kind="ExternalInput").ap(),
        "b3": nc.dram_tensor("b3", [128, 4], F32, kind="ExternalInput").ap(),
        "outT": nc.dram_tensor("outT", [E, S], BF16, kind="ExternalOutput").ap(),
    }
    with tile.TileContext(nc) as tc:
        for _ in range(reps):
            _emit(nc, tc, io)
    nc.compile()
    return nc


def make_in_maps(q, k, v, Wq, bq, Wk, bk, Wv, bv, Wo, bo):
    q, k, v = (np.asarray(x, np.float32) for x in (q, k, v))
    Wq, Wk, Wv, Wo = (np.asarray(x, np.float32) for x in (Wq, Wk, Wv, Wo))
    bq, bk, bv = (np.asarray(x, np.float32) for x in (bq, bk, bv))
    xT = {b: {n: np.ascontiguousarray(a[b].T).astype(NPBF16)
              for n, a in (("xqT", q), ("xkT", k), ("xvT", v))}
          for b in range(2)}
    in_maps = []
    for c in range(8):
        b, g = divmod(c, 4)
        cs = slice(g * HC, (g + 1) * HC)
        b3 = np.stack([bq[cs].reshape(2, 128)[0], bq[cs].reshape(2, 128)[1],
                       bk[cs].reshape(2, 128)[0], bk[cs].reshape(2, 128)[1]],
                      axis=1)
        in_maps.append({
            "xqT": xT[b]["xqT"], "xkT": xT[b]["xkT"], "xvT": xT[b]["xvT"],
            "wq": np.ascontiguousarray(Wq[:, cs]).astype(NPBF16),
            "wk": np.ascontiguousarray(Wk[:, cs]).astype(NPBF16),
            "wv": np.ascontiguousarray(Wv[:, cs]).astype(NPBF16),
            "wo": np.ascontiguousarray(Wo[cs, :]).astype(NPBF16),
            "b3": np.ascontiguousarray(b3, np.float32),
        })
    return in_maps


def kernel(q, k, v, Wq, bq, Wk, bk, Wv, bv, Wo, bo):
    if "nc" not in _CACHE:
        _CACHE["nc"] = build_program()
    nc = _CACHE["nc"]
    in_maps = make_in_maps(q, k, v, Wq, bq, Wk, bk, Wv, bv, Wo, bo)
    res = run_bass_kernel_spmd(nc, in_maps, list(range(8))).results
    bo = np.asarray(bo, np.float32)
    bv = np.asarray(bv, np.float32)
    Wo = np.asarray(Wo, np.float32)
    extra = bv @ Wo + bo  # bv folds through the output projection
    out = np.empty((2, S, E), np.float32)
    for b in range(2):
        acc = res[4 * b]["outT"].astype(np.float32)
        for g in range(1, 4):
            acc += res[4 * b + g]["outT"].astype(np.float32)
        out[b] = acc.T + extra
    return out
